# revision 21
# baseline (speedup 1.0000x reference)
"""Pairwise squared-Euclidean distance map on 8 TRN2 NeuronCores.

d[b, i, j] = sum_c (a[b, c, i] - b[b, c, j])^2
           = aa[b, i] + bb[b, j] - 2 * <a[b, :, i], b[b, :, j]>

Sharding: data-parallel over the N dimension (rows of the distance map).
Core k computes d[:, k*512:(k+1)*512, :] from a[:, :, k*512:(k+1)*512]
and the full (small) b tensor.

Per-core kernel: one augmented fp16 matmul per output tile computes the
full expression in a single PE pass (16-bit operands stream 1
column/cycle vs ~3 for fp32r on TRN2; fp16 keeps 10 mantissa bits). The
augmentation (K = C+4 = 68 contraction rows, pairing lhsT row k with
rhs row k) is
    lhsT = [ -2*a ;   1   ;   0     ; aa-64 ; 64 ]   (128 N cols)
    rhs  = [   b  ;  bb   ; scratch ;   1   ;  1 ]   (512 M cols)
so lhsT.T @ rhs = -2*a.b + bb + (aa-64) + 64 = d, accumulated in fp32
PSUM. aa is centered (E[|a_i|^2] = C = 64) so its fp16 rounding error
shrinks ~4x. The PSUM tile is drained to SBUF as fp16 (d is O(100),
well inside fp16 range; total rel err ~6e-4) which HALVES the dominant
cost of the kernel — the 256 MB of distance-map stores — and is upcast
to fp32 on the host.

Norm rows are produced by selector matmuls over squared inputs. For bb,
the squares of the two M-halves are stacked on the 128 partitions, so
one [128, 2] all-ones-block selector pass yields bb for TWO output
columns per streamed column (halving the norm matmul columns); the
[2, 512] PSUM pair is copied into rhs rows 64:66 (bb-low lands in
place; the zero-weighted scratch row stages bb-high, relocated by an
SBUF->SBUF DMA — DMAs may start at any partition while compute-engine
access patterns may only start at partitions {0, 32, 64, 96}). The a
side appends a ones row at partition 64 and one [65, 4] selector pass
emits [ones ; zero ; aa-64 ; 64] directly.

Scheduling: drain copies and input squaring alternate between the
Vector and Scalar engines; input DMA triggers ride the SWDGE (gpsimd)
path so the sync HWDGE ring carries only stores; batch bt+1's prep is
emitted interleaved with batch bt's row-block mains so no engine sees a
long serial prep chain; every [128, 1024] PSUM tile is stored as its
own 256 KB DMA so stores flow from a few microseconds into the kernel.
Measured: ~91 us HW exec on 8 cores (vs ~52 us HBM store floor plus
~16 us fixed preamble/barrier overhead; the PE's throttled 1.2 GHz
clock costs ~62 us of matmul streaming, overlapped with the stores).
"""

import numpy as np
from contextlib import ExitStack

import concourse.bass as bass
import concourse.bacc as bacc
import concourse.mybir as mybir
from concourse.tile import TileContext
from concourse.bass_utils import run_bass_kernel_spmd

B, C, N, M = 4, 64, 4096, 4096
NCORES = 8
NSH = N // NCORES          # 512 N rows per core
NB = NSH // 128            # 4 row blocks of 128
MC = 512                   # matmul moving free dim (one PSUM bank of fp32)
PSUM_W = 1024              # main PSUM tile width (2 banks, 2 matmuls)
KAUG = C + 4               # contraction dim with the norm/const rows

F32 = mybir.dt.float32
F16 = mybir.dt.float16

_CACHE = {}


def _build_nc():
    nc = bacc.Bacc(
        "TRN2",
        target_bir_lowering=False,
        debug=False,
        enable_asserts=True,
        num_devices=NCORES,
    )
    a_d = nc.declare_dram_parameter("a", [B, C, NSH], F32, isOutput=False)
    b_d = nc.declare_dram_parameter("b", [B, C, M], F16, isOutput=False)
    ones_d = nc.declare_dram_parameter("ones", [M], F16, isOutput=False)
    # selector columns (see _make_sel): per side [pick-ones | sum-sq - 64 |
    # 128 * pick-ones] arranged for the row order each side needs
    sel_d = nc.declare_dram_parameter("sel", [C + 1, 7], F16, isOutput=False)
    # two-way stacked selector: col0 sums partitions 0:64, col1 sums 64:128
    sel2_d = nc.declare_dram_parameter("sel2", [2 * C, 2], F16, isOutput=False)
    # constant init for baug rows 65:68 (scratch + two ones rows)
    bones_d = nc.declare_dram_parameter("bones", [3, M], F16, isOutput=False)
    d_d = nc.declare_dram_parameter("d", [B, NSH, M], F16, isOutput=True)

    with ExitStack() as ctx:
        tc = ctx.enter_context(TileContext(nc))
        const = ctx.enter_context(tc.tile_pool(name="const", bufs=1))
        bpool = ctx.enter_context(tc.tile_pool(name="baug", bufs=3))
        apool = ctx.enter_context(tc.tile_pool(name="aaug", bufs=3))
        rawp = ctx.enter_context(tc.tile_pool(name="araw", bufs=3))
        sqbp = ctx.enter_context(tc.tile_pool(name="sqb", bufs=3))
        sqap = ctx.enter_context(tc.tile_pool(name="sqa", bufs=3))
        stage = ctx.enter_context(tc.tile_pool(name="stage", bufs=10))
        mpsum = ctx.enter_context(tc.tile_pool(name="mpsum", bufs=3, space="PSUM"))
        xpsum = ctx.enter_context(tc.tile_pool(name="xpsum", bufs=2, space="PSUM"))

        sel = const.tile([C + 1, 7], F16)
        nc.gpsimd.dma_start(out=sel[:, :], in_=sel_d[:, :])
        sel2 = const.tile([2 * C, 2], F16)
        nc.gpsimd.dma_start(out=sel2[:, :], in_=sel2_d[:, :])

        state = {"copy_tick": 0}
        CHUNK = M // NB  # 1024 cols of b prepped per unit; NB units per batch

        def alt_copy(dst, src):
            if state["copy_tick"] % 2 == 0:
                nc.vector.tensor_copy(dst, src)
            else:
                nc.scalar.copy(dst, src)
            state["copy_tick"] += 1


        def prep_a(bt):
            """lhsT a_aug [C+3, NSH] = [-2a ; aa-64 ; 1 ; 128] (fp16)."""
            araw = rawp.tile([C, NSH], F32, tag="araw", name=f"araw{bt}")
            nc.gpsimd.dma_start(out=araw[:, :], in_=a_d[bt])
            aaug = apool.tile([KAUG, NSH], F16, tag="aaug", name=f"aaug{bt}")
            nc.vector.tensor_scalar_mul(aaug[0:C, :], araw[:, :], -2.0)
            sqa = sqap.tile([C + 1, NSH], F16, tag="sqa", name=f"sqa{bt}")
            nc.vector.tensor_mul(sqa[0:C, :], araw[:, :], araw[:, :])
            nc.gpsimd.dma_start(out=sqa[C : C + 1, :], in_=ones_d[0:NSH][None, :])
            pa = xpsum.tile([4, NSH], F32, tag="xp", name=f"pa{bt}")
            nc.tensor.matmul(pa[:, :], sel[:, 3:7], sqa[:, :])
            nc.vector.tensor_copy(aaug[C : C + 4, :], pa[:, :])
            return aaug

        def prep_b_alloc(bt):
            baug = bpool.tile([KAUG, M], F16, tag="baug", name=f"baug{bt}")
            # squares of b cols [0:M/2) on partitions 0:64, cols [M/2:M) on
            # 64:128 - one selector pass covers two columns of bb at once
            sqb = sqbp.tile([2 * C, M // 2], F16, tag="sqb", name=f"sqb{bt}")
            nc.gpsimd.dma_start(out=baug[C + 1 : C + 4, :], in_=bones_d[:, :])
            return baug, sqb

        def prep_b_chunk(bt, q, baug, sqb):
            """Load+square b cols [q*CHUNK, (q+1)*CHUNK); after both halves
            of a column pair are in, emit the bb selector matmuls."""
            c0, c1 = q * CHUNK, (q + 1) * CHUNK
            half = M // 2
            nc.gpsimd.dma_start(out=baug[0:C, c0:c1], in_=b_d[bt][:, c0:c1])
            if c0 < half:
                dst = sqb[0:C, c0:c1]
            else:
                dst = sqb[C : 2 * C, c0 - half : c1 - half]
            if q % 2 == 0:
                nc.vector.tensor_mul(dst, baug[0:C, c0:c1], baug[0:C, c0:c1])
            else:
                nc.scalar.square(dst, baug[0:C, c0:c1])
            if c0 >= half:  # paired halves [c0-half, c1-half) now complete
                for j in range((c0 - half) // MC, (c1 - half) // MC):
                    pb = xpsum.tile([4, MC], F32, tag="xp", name=f"pb{bt}_{j}")
                    nc.tensor.matmul(
                        pb[0:2, :], sel2[:, :], sqb[:, j * MC : (j + 1) * MC]
                    )
                    # row C gets bb-low in place; row C+1 (zero-weighted
                    # scratch) stages bb-high, relocated by a DMA (DMAs may
                    # start at any partition; engine APs may not)
                    alt_copy(
                        baug[C : C + 2, j * MC : (j + 1) * MC], pb[0:2, :]
                    )
                    nc.gpsimd.dma_start(
                        out=baug[C : C + 1, half + j * MC : half + (j + 1) * MC],
                        in_=baug[C + 1 : C + 2, j * MC : (j + 1) * MC],
                    )

        def mains(bt, i, aaug, baug):
            """One 128-row output block: 8 matmuls, 4 drain+store chunks."""
            for jj in range(M // PSUM_W):
                pt = mpsum.tile(
                    [128, PSUM_W], F32, tag="mp", name=f"mp{bt}_{i}_{jj}"
                )
                for h in range(PSUM_W // MC):
                    col = jj * PSUM_W + h * MC
                    nc.tensor.matmul(
                        pt[:, h * MC : (h + 1) * MC],
                        aaug[:, i * 128 : (i + 1) * 128],
                        baug[:, col : col + MC],
                    )
                st = stage.tile(
                    [128, PSUM_W], F16, tag="st", name=f"st{bt}_{i}_{jj}"
                )
                alt_copy(st[:, :], pt[:, :])
                nc.sync.dma_start(
                    out=d_d[
                        bt,
                        i * 128 : (i + 1) * 128,
                        jj * PSUM_W : (jj + 1) * PSUM_W,
                    ],
                    in_=st[:, :],
                )

        # batch 0 prep up front; batch bt+1's prep units interleave with
        # batch bt's row-block mains so no engine sees a long serial chain
        aaug_t = prep_a(0)
        baug_t, sqb_t = prep_b_alloc(0)
        for q in (0, 2, 1, 3):
            prep_b_chunk(0, q, baug_t, sqb_t)
        for bt in range(B):
            for i in range(NB):
                mains(bt, i, aaug_t, baug_t)
                if bt + 1 < B and i < 2:
                    if i == 0:
                        naaug = prep_a(bt + 1)
                        nbaug, nsqb = prep_b_alloc(bt + 1)
                    # two prep units per i-block so the last bb rows land
                    # well before the next batch's mains need them
                    for q in ((0, 2), (1, 3))[i]:
                        prep_b_chunk(bt + 1, q, nbaug, nsqb)
            if bt + 1 < B:
                aaug_t, baug_t, sqb_t = naaug, nbaug, nsqb

    nc.compile()
    return nc


def _get_nc():
    if "nc" not in _CACHE:
        _CACHE["nc"] = _build_nc()
    return _CACHE["nc"]


def _make_sel():
    sel = np.zeros([C + 1, 7], dtype=np.float32)
    # a side -> aaug rows [C..C+3] = [ones ; zero ; aa-64 ; 64]
    sel[C, 3] = 1.0
    # col 4: all zero (pairs the bb scratch row)
    sel[0:C, 5] = 1.0
    sel[C, 5] = -64.0
    sel[C, 6] = 64.0
    return sel


def _make_sel2():
    sel2 = np.zeros([2 * C, 2], dtype=np.float32)
    sel2[0:C, 0] = 1.0
    sel2[C : 2 * C, 1] = 1.0
    return sel2


def _make_in_maps(a, b):
    a = np.ascontiguousarray(np.asarray(a, dtype=np.float32))
    b = np.ascontiguousarray(np.asarray(b, dtype=np.float32)).astype(np.float16)
    ones = np.ones([M], dtype=np.float16)
    sel = _make_sel().astype(np.float16)
    sel2 = _make_sel2().astype(np.float16)
    bones = np.ones([3, M], dtype=np.float16)
    in_maps = []
    for k in range(NCORES):
        in_maps.append(
            {
                "a": np.ascontiguousarray(a[:, :, k * NSH : (k + 1) * NSH]),
                "b": b,
                "ones": ones,
                "sel": sel,
                "sel2": sel2,
                "bones": bones,
            }
        )
    return in_maps


def kernel(a, b, _trace=False, _trace_kwargs=None):
    nc = _get_nc()
    in_maps = _make_in_maps(a, b)
    res = run_bass_kernel_spmd(
        nc,
        in_maps,
        core_ids=list(range(NCORES)),
        trace=_trace,
        **(_trace_kwargs or {}),
    )
    out = np.concatenate(
        [res.results[k]["d"] for k in range(NCORES)], axis=1
    ).astype(np.float32)
    if _trace:
        _CACHE["last_results"] = res
    return out
